# revision 12
# baseline (speedup 1.0000x reference)
"""Trainium2 8-core Bass kernel for AdaptiveAttentionTransformerBlock.

Sparse attention with a latent (stride-64 mean-pooled KV) branch and a local
sliding-window (width 64) branch, concatenated per head and mixed by w_mix.

Sharding: 16 heads -> 2 heads per core (all 8 cores), both batches per core.
Each core computes qkv + RoPE + both attention branches for its 2 heads in a
transposed [feature, seq] layout, writes per-destination output shards, and a
single 8-rank AllToAll redistributes from head-sharding to (batch, seq/4)
sharding. Each core then computes the full mix projection for its 512-token
slice; the host concatenates the 8 slices.

Self-contained: hardcodes shapes B=2, S=2048, E=1024, H=16, D=64, stride=64.
"""

import os
import sys

import numpy as np

for _p in ("/opt/trn_rl_repo",):
    if _p not in sys.path and os.path.isdir(_p):
        sys.path.insert(0, _p)

import ml_dtypes  # noqa: E402

B, S, E, H, D = 2, 2048, 1024, 16, 64
STRIDE, C = 64, 32
NCORES, HLOC = 8, 2
SB = S // 4  # 512, per-core output seq slice
BF16 = ml_dtypes.bfloat16

_BUILD_CACHE = {}


# ----------------------------------------------------------------------------
# host-side constants
# ----------------------------------------------------------------------------

def _host_constants():
    inv = 1.0 / (10000.0 ** (np.arange(0, D, 2, dtype=np.float64) / D))
    pos = np.arange(S, dtype=np.float64)
    ang = pos[None, :] * inv[(np.arange(D) % (D // 2))][:, None]  # [64, S]
    cosT = np.cos(ang).astype(np.float32)
    sinT = np.sin(ang).astype(np.float32)
    cos2 = np.vstack([cosT, cosT])  # [128, S] f32
    sin2 = np.vstack([sinT, sinT])

    L = np.zeros((D, D), np.float32)
    for j in range(D // 2):
        L[2 * j + 1, 2 * j] = -1.0
        L[2 * j, 2 * j + 1] = 1.0
    L2 = np.zeros((2 * D, 2 * D), np.float32)
    L2[:D, :D] = L
    L2[D:, D:] = L

    i = np.arange(128)
    mdiag = ((i[None, :] >= i[:, None]) & (i[None, :] - i[:, None] <= 63)).astype(np.float32)
    mprev = (i[:, None] >= i[None, :] + 65).astype(np.float32)

    cc = np.arange(C)
    ss = np.arange(S)
    latm = (cc[:, None] * STRIDE <= ss[None, :]).astype(np.float32)  # [32, S]

    A = np.zeros((S, C), np.float32)
    for c in range(C):
        A[c * STRIDE:(c + 1) * STRIDE, c] = 1.0 / STRIDE

    ident = np.eye(128, dtype=np.float32)
    return {
        "cos2": cos2,
        "sin2": sin2,
        "r2": L2.astype(BF16),
        "mprev": mprev.astype(BF16),
        "mdiag": mdiag.astype(BF16),
        "latm": latm.astype(BF16),
        "amat": A.astype(BF16),
        "ident": ident,
    }


# ----------------------------------------------------------------------------
# bass graph
# ----------------------------------------------------------------------------

def build_bass():
    import concourse.bass as bass
    import concourse.mybir as mybir
    import concourse.tile as tile
    from concourse import bacc

    f32 = mybir.dt.float32
    bf16 = mybir.dt.bfloat16

    nc = bacc.Bacc(
        "TRN2",
        target_bir_lowering=False,
        debug=False,
        num_devices=NCORES,
    )

    xT_d = nc.declare_dram_parameter("xT", [E, B * S], bf16, isOutput=False)
    wqkT_d = nc.declare_dram_parameter("wqkT", [E, 256], bf16, isOutput=False)
    wvT_d = nc.declare_dram_parameter("wvT", [E, 128], bf16, isOutput=False)
    wmixT_d = nc.declare_dram_parameter("wmixT", [2 * E, E], bf16, isOutput=False)
    cos2_d = nc.declare_dram_parameter("cos2", [128, S], f32, isOutput=False)
    sin2_d = nc.declare_dram_parameter("sin2", [128, S], f32, isOutput=False)
    r2_d = nc.declare_dram_parameter("r2", [128, 128], bf16, isOutput=False)
    mprev_d = nc.declare_dram_parameter("mprev", [128, 128], bf16, isOutput=False)
    mdiag_d = nc.declare_dram_parameter("mdiag", [128, 128], bf16, isOutput=False)
    latm_d = nc.declare_dram_parameter("latm", [C, S], bf16, isOutput=False)
    amat_d = nc.declare_dram_parameter("amat", [S, C], bf16, isOutput=False)
    ident_d = nc.declare_dram_parameter("ident", [128, 128], f32, isOutput=False)
    y_d = nc.declare_dram_parameter("y", [SB, E], f32, isOutput=True)

    SCALE = 1.0 / 8.0  # 1/sqrt(D)

    with tile.TileContext(nc, num_cores=NCORES) as tc:
        with (
            tc.tile_pool(name="const", bufs=1) as constp,
            tc.tile_pool(name="xt", bufs=1) as xtp,
            tc.tile_pool(name="qk", bufs=2) as qkp,
            tc.tile_pool(name="vex", bufs=2) as vexp,
            tc.tile_pool(name="lat", bufs=2) as latp,
            tc.tile_pool(name="plat", bufs=3) as platp,
            tc.tile_pool(name="tmp", bufs=4) as tmpp,
            tc.tile_pool(name="oc", bufs=4) as ocp,
            tc.tile_pool(name="ps", bufs=8, space="PSUM") as psp,
            tc.tile_pool(name="dram", bufs=1, space="DRAM") as dramp,
        ):
            # ---- persistent loads ----
            wqk_sb = constp.tile([128, 8, 256], bf16, name="wqk_sb")
            nc.sync.dma_start(wqk_sb[:], wqkT_d.ap().rearrange("(eo p) j -> p eo j", p=128))
            wv_sb = constp.tile([128, 8, 128], bf16, name="wv_sb")
            nc.sync.dma_start(wv_sb[:], wvT_d.ap().rearrange("(eo p) j -> p eo j", p=128))
            cos_sb = constp.tile([128, S], f32, name="cos_sb")
            nc.sync.dma_start(cos_sb[:], cos2_d.ap())
            sin_sb = constp.tile([128, S], f32, name="sin_sb")
            nc.sync.dma_start(sin_sb[:], sin2_d.ap())
            r2_sb = constp.tile([128, 128], bf16, name="r2_sb")
            nc.sync.dma_start(r2_sb[:], r2_d.ap())
            mprev_sb = constp.tile([128, 128], bf16, name="mprev_sb")
            nc.sync.dma_start(mprev_sb[:], mprev_d.ap())
            mdiag_sb = constp.tile([128, 128], bf16, name="mdiag_sb")
            nc.sync.dma_start(mdiag_sb[:], mdiag_d.ap())
            latm_sb = constp.tile([C, S], bf16, name="latm_sb")
            nc.sync.dma_start(latm_sb[:], latm_d.ap())
            amat_sb = constp.tile([128, 16, C], bf16, name="amat_sb")
            nc.sync.dma_start(amat_sb[:], amat_d.ap().rearrange("(t p) c -> p t c", p=128))
            ident_sb = constp.tile([128, 128], f32, name="ident_sb")
            nc.sync.dma_start(ident_sb[:], ident_d.ap())

            # x^T, per 512-column chunk (8 chunks across both batches)
            xt_tiles = []
            for i in range(8):
                t = xtp.tile([128, 8, 512], bf16, name=f"xt{i}", tag=f"xt{i}")
                nc.sync.dma_start(
                    t[:],
                    xT_d.ap()
                    .rearrange("(eo p) s -> p eo s", p=128)[:, :, i * 512:(i + 1) * 512],
                )
                xt_tiles.append(t)

            wmix_sb = constp.tile([128, 16, E], bf16, name="wmix_sb")
            nc.sync.dma_start(wmix_sb[:], wmixT_d.ap().rearrange("(fo p) e -> p fo e", p=128))

            a2a_in = dramp.tile([NCORES, 256, SB], bf16, name="a2a_in")
            a2a_out = dramp.tile([NCORES, 256, SB], bf16, name="a2a_out")

            # ---- per-batch compute ----
            for b in range(B):
                q_rot = qkp.tile([128, S], bf16, name=f"q_rot{b}", tag="q_rot")
                k_rot = qkp.tile([128, S], bf16, name=f"k_rot{b}", tag="k_rot")
                # qk projection + rope, per 512-token chunk
                for jt in range(2):  # 0: q, 1: k
                    dst = q_rot if jt == 0 else k_rot
                    for sc in range(4):
                        xt = xt_tiles[b * 4 + sc]
                        ps_qk = psp.tile([128, 512], f32, name=f"ps_qk{b}{jt}{sc}", tag="ps")
                        for e in range(8):
                            nc.tensor.matmul(
                                ps_qk[:],
                                wqk_sb[:, e, jt * 128:(jt + 1) * 128],
                                xt[:, e, :],
                                start=(e == 0),
                                stop=(e == 7),
                            )
                        tmp_bf = tmpp.tile([128, 512], bf16, name="tmp_bf", tag="tmp_bf")
                        nc.any.tensor_copy(tmp_bf[:], ps_qk[:])
                        ps_rh = psp.tile([128, 512], f32, name=f"ps_rh{b}{jt}{sc}", tag="ps")
                        nc.tensor.matmul(ps_rh[:], r2_sb[:], tmp_bf[:], start=True, stop=True)
                        ssl = slice(sc * 512, (sc + 1) * 512)
                        t1 = tmpp.tile([128, 512], f32, name="t1", tag="t1")
                        nc.vector.tensor_mul(t1[:], ps_qk[:], cos_sb[:, ssl])
                        t2 = tmpp.tile([128, 512], f32, name="t2", tag="t2")
                        nc.vector.tensor_mul(t2[:], ps_rh[:], sin_sb[:, ssl])
                        nc.vector.tensor_add(dst[:, ssl], t1[:], t2[:])

                # v projection -> v_ext [s-part, 16, 2 heads, 65] with ones col
                v_ext = vexp.tile([128, 16, HLOC, 65], bf16, name=f"v_ext{b}", tag="v_ext")
                nc.any.memset(v_ext[:, :, :, 64], 1.0)
                for st in range(16):
                    xt = xt_tiles[b * 4 + st // 4]
                    ps_v = psp.tile([128, 128], f32, name=f"ps_v{b}{st}", tag="ps")
                    for e in range(8):
                        nc.tensor.matmul(
                            ps_v[:],
                            xt[:, e, (st % 4) * 128:(st % 4 + 1) * 128],
                            wv_sb[:, e, :],
                            start=(e == 0),
                            stop=(e == 7),
                        )
                    for hh in range(HLOC):
                        nc.any.tensor_copy(
                            v_ext[:, st, hh, 0:64], ps_v[:, hh * 64:(hh + 1) * 64]
                        )

                # latent KV: k chunk-means for both heads at their partition offsets
                import concourse.mybir as _mb
                klat2 = latp.tile([128, C], bf16, name=f"klat2{b}", tag="klat2")
                kl32 = latp.tile([128, C], f32, name=f"kl32{b}", tag="kl32")
                nc.vector.tensor_reduce(
                    out=kl32[:],
                    in_=k_rot[:].rearrange("p (c w) -> p c w", w=STRIDE),
                    op=_mb.AluOpType.add,
                    axis=_mb.AxisListType.X,
                )
                nc.scalar.activation(klat2[:], kl32[:], _mb.ActivationFunctionType.Copy,
                                     bias=0.0, scale=1.0 / STRIDE)
                vlat = []
                for hh in range(HLOC):
                    ps_vl = psp.tile([C, 65], f32, name=f"ps_vl{b}{hh}", tag="ps")
                    for st in range(16):
                        nc.tensor.matmul(
                            ps_vl[:],
                            amat_sb[:, st, :],
                            v_ext[:, st, hh, :],
                            start=(st == 0),
                            stop=(st == 15),
                        )
                    vl = latp.tile([C, 65], bf16, name=f"vlat{b}{hh}", tag=f"vlat{hh}")
                    nc.any.tensor_copy(vl[:], ps_vl[:])
                    vlat.append(vl)

                # attention per head
                import concourse.mybir as mb
                for hh in range(HLOC):
                    hsl = slice(hh * D, (hh + 1) * D)
                    for qc in range(4):
                        qsl512 = slice(qc * 512, (qc + 1) * 512)
                        ps_ls = psp.tile([C, 512], f32, name=f"ps_ls{b}{hh}{qc}", tag="ps")
                        nc.tensor.matmul(ps_ls[:], klat2[hsl, :], q_rot[hsl, qsl512],
                                         start=True, stop=True)
                        p_lat = platp.tile([C, 512], bf16, name="p_lat", tag="p_lat")
                        nc.scalar.activation(p_lat[:], ps_ls[:],
                                             mb.ActivationFunctionType.Exp,
                                             bias=0.0, scale=SCALE)
                        nc.vector.tensor_mul(p_lat[:], p_lat[:], latm_sb[:, qsl512])

                        for qi in range(4):
                            qt = qc * 4 + qi
                            qsl = slice(qt * 128, (qt + 1) * 128)
                            # local scores (transposed [k, q]): prev + diag key tiles
                            ps_sd = psp.tile([128, 128], f32, name="ps_sd", tag="ps")
                            nc.tensor.matmul(
                                ps_sd[:],
                                k_rot[hsl, qsl],
                                q_rot[hsl, qsl],
                                start=True, stop=True,
                            )
                            p_d = tmpp.tile([128, 128], bf16, name="p_d", tag="p_d")
                            nc.scalar.activation(p_d[:], ps_sd[:],
                                                 mb.ActivationFunctionType.Exp,
                                                 bias=0.0, scale=SCALE)
                            nc.vector.tensor_mul(p_d[:], p_d[:], mdiag_sb[:])
                            if qt > 0:
                                ps_sp = psp.tile([128, 128], f32, name="ps_sp", tag="ps")
                                nc.tensor.matmul(
                                    ps_sp[:],
                                    k_rot[hsl, (qt - 1) * 128:qt * 128],
                                    q_rot[hsl, qsl],
                                    start=True, stop=True,
                                )
                                p_p = tmpp.tile([128, 128], bf16, name="p_p", tag="p_p")
                                nc.scalar.activation(p_p[:], ps_sp[:],
                                                     mb.ActivationFunctionType.Exp,
                                                     bias=0.0, scale=SCALE)
                                nc.vector.tensor_mul(p_p[:], p_p[:], mprev_sb[:])

                            ps_av = psp.tile([128, 65], f32, name="ps_av", tag="ps")
                            if qt > 0:
                                nc.tensor.matmul(ps_av[:], p_p[:],
                                                 v_ext[:, qt - 1, hh, :],
                                                 start=True, stop=False)
                                nc.tensor.matmul(ps_av[:], p_d[:],
                                                 v_ext[:, qt, hh, :],
                                                 start=False, stop=True)
                            else:
                                nc.tensor.matmul(ps_av[:], p_d[:],
                                                 v_ext[:, qt, hh, :],
                                                 start=True, stop=True)

                            ps_avl = psp.tile([128, 65], f32, name="ps_avl", tag="ps")
                            nc.tensor.matmul(ps_avl[:], p_lat[:, qi * 128:(qi + 1) * 128],
                                             vlat[hh][:], start=True, stop=True)

                            # normalize -> ocat [128 q, 128 (cmp|loc)] f32
                            rec_c = tmpp.tile([128, 1], f32, name="rec_c", tag="rec_c")
                            nc.vector.reciprocal(rec_c[:], ps_avl[:, 64:65])
                            rec_l = tmpp.tile([128, 1], f32, name="rec_l", tag="rec_l")
                            nc.vector.reciprocal(rec_l[:], ps_av[:, 64:65])
                            ocat = ocp.tile([128, 128], f32, name="ocat", tag="ocat")
                            nc.vector.tensor_scalar_mul(ocat[:, 0:64], ps_avl[:, 0:64], rec_c[:])
                            nc.vector.tensor_scalar_mul(ocat[:, 64:128], ps_av[:, 0:64], rec_l[:])

                            # transpose -> [128 F, 128 q] and stage to a2a_in
                            ps_t = psp.tile([128, 128], f32, name="ps_t", tag="ps")
                            nc.tensor.transpose(ps_t[:], ocat[:], ident_sb[:])
                            oT = ocp.tile([128, 128], bf16, name="oT", tag="oT")
                            nc.any.tensor_copy(oT[:], ps_t[:])
                            dshard = b * 4 + qt // 4
                            nc.sync.dma_start(
                                a2a_in[dshard, hh * 128:(hh + 1) * 128,
                                       (qt % 4) * 128:(qt % 4 + 1) * 128],
                                oT[:],
                            )

            # ---- all-to-all: head-shard -> (batch, seq/4)-shard ----
            import concourse.mybir as mb2
            nc.gpsimd.collective_compute(
                "AllToAll",
                mb2.AluOpType.bypass,
                replica_groups=[list(range(NCORES))],
                ins=[a2a_in[:].opt()],
                outs=[a2a_out[:].opt()],
            )

            # ---- mix projection for this core's 512-token slice ----
            gat_sb = constp.tile([128, 16, SB], bf16, name="gat_sb")
            nc.sync.dma_start(
                gat_sb[:],
                a2a_out[:].rearrange("g (fi p) s -> p (g fi) s", p=128),
            )
            for st in range(4):
                for ec in range(2):
                    ps_y = psp.tile([128, 512], f32, name=f"ps_y{st}{ec}", tag="ps")
                    for fo in range(16):
                        nc.tensor.matmul(
                            ps_y[:],
                            gat_sb[:, fo, st * 128:(st + 1) * 128],
                            wmix_sb[:, fo, ec * 512:(ec + 1) * 512],
                            start=(fo == 0),
                            stop=(fo == 15),
                        )
                    y_sb = tmpp.tile([128, 512], f32, name="y_sb", tag="y_sb")
                    nc.any.tensor_copy(y_sb[:], ps_y[:])
                    nc.sync.dma_start(
                        y_d.ap()[st * 128:(st + 1) * 128, ec * 512:(ec + 1) * 512],
                        y_sb[:],
                    )

    nc.compile()
    return nc


# ----------------------------------------------------------------------------
# host wrapper
# ----------------------------------------------------------------------------

def _numpy_reference(x, w_qkv, w_mix, stride):
    """Fallback for unexpected shapes/stride: direct numpy port of the reference."""
    x = np.asarray(x, np.float32)
    Bx, Sx, Ex = x.shape
    Hx = 16
    Dx = Ex // Hx
    stride = int(stride)
    qkv = x @ np.asarray(w_qkv, np.float32).T
    qkv = qkv.reshape(Bx, Sx, 3, Hx, Dx).transpose(2, 0, 3, 1, 4)
    q, k, v = qkv[0], qkv[1], qkv[2]
    inv = 1.0 / (10000.0 ** (np.arange(0, Dx, 2, dtype=np.float32) / Dx))
    pos = np.arange(Sx, dtype=np.float32)
    emb = np.concatenate([pos[:, None] * inv[None, :]] * 2, axis=-1)
    cos, sin = np.cos(emb)[None, None], np.sin(emb)[None, None]

    def rot(t):
        t1 = t[..., ::2]
        t2 = t[..., 1::2]
        return np.stack((-t2, t1), axis=-1).reshape(t.shape)

    q = q * cos + rot(q) * sin
    k = k * cos + rot(k) * sin

    def compress(t):
        chunks = -(-Sx // stride)
        pad = chunks * stride - Sx
        if pad:
            t = np.concatenate([t, np.broadcast_to(t[:, :, -1:, :], t.shape[:2] + (pad, Dx))], axis=2)
        return t.reshape(Bx, Hx, chunks, stride, Dx).mean(axis=3)

    scale = 1.0 / np.sqrt(Dx)
    k_lat, v_lat = compress(k), compress(v)
    Cx = k_lat.shape[2]
    sc = np.einsum("bhsd,bhcd->bhsc", q, k_lat) * scale
    chunk_start = np.minimum(np.arange(Cx) * stride, Sx - 1)
    qpos = np.arange(Sx)
    mask = chunk_start[None, :] > qpos[:, None]
    sc = np.where(mask[None, None], -np.inf, sc)
    sc = sc - sc.max(axis=-1, keepdims=True)
    w = np.exp(sc)
    w = w / w.sum(axis=-1, keepdims=True)
    w = np.nan_to_num(w)
    compressed = np.einsum("bhsc,bhcd->bhsd", w, v_lat)

    kpos = np.arange(Sx)
    blocked = (kpos[None, :] > qpos[:, None]) | (kpos[None, :] < qpos[:, None] - stride + 1)
    ls = np.einsum("bhsd,bhtd->bhst", q, k) * scale
    ls = np.where(blocked[None, None], -np.inf, ls)
    ls = ls - ls.max(axis=-1, keepdims=True)
    lw = np.exp(ls)
    lw = lw / lw.sum(axis=-1, keepdims=True)
    local = np.einsum("bhst,bhtd->bhsd", lw, v)
    out = np.concatenate([compressed, local], axis=-1)
    out = out.transpose(0, 2, 1, 3).reshape(Bx, Sx, 2 * Ex)
    return (out @ np.asarray(w_mix, np.float32).T).astype(np.float32)


def _make_in_maps(x, w_qkv, w_mix):
    consts = _host_constants()
    xT = np.ascontiguousarray(
        np.concatenate([x[0].T, x[1].T], axis=1)
    ).astype(BF16)
    wmixT = np.ascontiguousarray(np.asarray(w_mix, np.float32).T).astype(BF16)
    in_maps = []
    for c in range(NCORES):
        h0 = HLOC * c
        rows_q = slice(h0 * D, (h0 + HLOC) * D)
        rows_k = slice(E + h0 * D, E + (h0 + HLOC) * D)
        rows_v = slice(2 * E + h0 * D, 2 * E + (h0 + HLOC) * D)
        wqkT = np.ascontiguousarray(
            np.concatenate([w_qkv[rows_q], w_qkv[rows_k]], axis=0).T
        ).astype(BF16)
        wvT = np.ascontiguousarray(w_qkv[rows_v].T).astype(BF16)
        in_maps.append({
            "xT": xT,
            "wqkT": wqkT,
            "wvT": wvT,
            "wmixT": wmixT,
            "cos2": consts["cos2"],
            "sin2": consts["sin2"],
            "r2": consts["r2"],
            "mprev": consts["mprev"],
            "mdiag": consts["mdiag"],
            "latm": consts["latm"],
            "amat": consts["amat"],
            "ident": consts["ident"],
        })
    return in_maps


def run_device(x, w_qkv, w_mix, trace=False, **spmd_kwargs):
    from concourse.bass_utils import run_bass_kernel_spmd

    if "nc" not in _BUILD_CACHE:
        _BUILD_CACHE["nc"] = build_bass()
    nc = _BUILD_CACHE["nc"]
    in_maps = _make_in_maps(np.asarray(x, np.float32), np.asarray(w_qkv, np.float32),
                            np.asarray(w_mix, np.float32))
    res = run_bass_kernel_spmd(nc, in_maps, core_ids=list(range(NCORES)),
                               trace=trace, **spmd_kwargs)
    out = np.zeros((B, S, E), np.float32)
    for d in range(NCORES):
        bb, sq = d // 4, d % 4
        out[bb, sq * SB:(sq + 1) * SB] = np.asarray(res.results[d]["y"], np.float32)
    return out, res


def kernel(x, w_qkv, w_mix, stride):
    x = np.asarray(x)
    if int(stride) != STRIDE or x.shape != (B, S, E):
        return _numpy_reference(x, w_qkv, w_mix, stride)
    out, _ = run_device(x, w_qkv, w_mix)
    return out
